# revision 33
# baseline (speedup 1.0000x reference)
"""GAT (2-layer, PyG-style) on 8 Trainium2 NeuronCores.

Strategy (matches the node/graph-parallel sharding hint):
  - Nodes partitioned into 8 contiguous ranges (6250/core); edges assigned to
    the core owning their DST node, sorted by dst, grouped per 128-dst window
    and padded to 128-edge tiles.
  - Node phase sharded: each core computes h_aug = x @ [W | W@att] for its own
    nodes; a chunked AllGather replicates the gather table (512-byte rows:
    256 fp8 features + 2H bf16 attention coefs + pad).
  - Edge phase per core: per-edge source rows fetched with the batched SWDGE
    dma_gather ucode (int16 indices, 16-partition wrapped); the table is split
    in two halves at the chunk boundary so indices fit int16. Attention
    logits assembled on-chip; segment softmax + scatter-add done as TensorE
    matmuls against host-built 0/1 matrices (onehot per (edge, dst-window)),
    with exp folded into the matmul rhs and 1/sum applied per-dst at the end.
  - Global mean-pool via matmul with a host-built node->graph 0/1 map,
    AllReduce of the [256, 50] partial, then the FC layer replicated.
"""

import os
import sys

sys.path.insert(0, "/opt/trn_rl_repo")

import numpy as np
import ml_dtypes

N_NODES, N_EDGES = 50000, 800000
IN_C, HID_C, OUT_C, HEADS = 256, 64, 256, 4
N_GRAPHS = 50
NEG_SLOPE = 0.2
NCORES = 8
WIN = 128         # dst nodes per aggregation window (psum partitions)
ROWB = 512        # gather-table row bytes: 256 fp8 feat + 2H bf16 a + pad
TBLC = 264        # max node-phase matmul output cols (256 h + 2H a-vals)
P = 128
CHUNK_TILES = 13  # node-tiles per AllGather chunk (4 chunks of 6250 rows)
SPLIT = NCORES * 2 * CHUNK_TILES * P  # 26624: table halves = 2 chunks each

BF16 = ml_dtypes.bfloat16

LAST_EXEC_NS = None  # set by kernel() when GAT_TRACE=1


# --------------------------------------------------------------------------
# host-side preprocessing
# --------------------------------------------------------------------------

def build_edge_data(src_rows, dst, n_nodes, ncores, win):
    """Per-core int16 gather indices + onehot matrices.

    Edges are ordered by (core, dst-window, lo/hi table half); each
    (window, half) run is padded to 128-edge tiles with a tile count shared
    across cores. Returns (tlo, thi, t0, percore) where percore[c] has
    idx16 [128, ttot*8] (16-partition-wrapped, replicated), oh_e/oh_d.
    """
    import ml_dtypes as _md
    nc_nodes = n_nodes // ncores
    nwin = (nc_nodes + win - 1) // win
    core_of = dst // nc_nodes
    win_of = (dst % nc_nodes) // win
    grp_of = (src_rows >= SPLIT).astype(np.int64)
    gid = (core_of * nwin + win_of) * 2 + grp_of
    order = np.argsort(gid, kind="stable")
    s_rows, s_dst = src_rows[order], dst[order]
    core_of, win_of, grp_of, gid = (a[order] for a in (core_of, win_of, grp_of, gid))

    counts = np.bincount(gid, minlength=ncores * nwin * 2).reshape(ncores, nwin, 2)
    twg = ((counts + P - 1) // P).max(axis=0)  # [nwin, 2] tiles, shared
    for w in range(nwin):
        if twg[w].sum() == 0:
            twg[w, 0] = 1
    tlo, thi = twg[:, 0].astype(np.int64), twg[:, 1].astype(np.int64)
    ntw = tlo + thi
    t0 = np.concatenate([[0], np.cumsum(ntw)])[:-1]
    ttot = int(ntw.sum())

    E = gid.shape[0]
    starts = np.concatenate([[0], np.cumsum(counts.ravel())])[:-1]
    k_in_g = np.arange(E) - starts[gid]
    tile_base = np.zeros((nwin, 2), dtype=np.int64)
    tile_base[:, 0] = t0
    tile_base[:, 1] = t0 + tlo
    gtile_all = tile_base[win_of, grp_of] + k_in_g // P
    lane = (k_in_g % P).astype(np.int64)
    dloc = (s_dst - (core_of * nc_nodes + win_of * win)).astype(np.int64)
    val = (s_rows - grp_of * SPLIT).astype(np.int16)

    percore = []
    for c in range(ncores):
        m = core_of == c
        slots = gtile_all[m] * P + lane[m]
        idx16 = np.zeros((16, ttot * 8), dtype=np.int16)
        idx16[slots % 16, slots // 16] = val[m]
        oh_e = np.zeros((ttot, P, win), dtype=np.float32)
        oh_d = np.zeros((ttot, win, P), dtype=np.float32)
        oh_e[gtile_all[m], lane[m], dloc[m]] = 1.0
        oh_d[gtile_all[m], dloc[m], lane[m]] = 1.0
        percore.append(dict(idx16=np.tile(idx16, (8, 1)),
                            oh_e=oh_e.astype(_md.float8_e4m3),
                            oh_d=oh_d.astype(_md.float8_e4m3)))
    return tlo, thi, t0, percore


def balance_nodes(dst, n_nodes, ncores, win):
    """Relabel nodes so each (core, window) bin carries a near-equal edge
    count: perm[old_id] = new_id. Greedy largest-degree-first into the
    lightest non-full bin. Bins follow the per-core window layout (the last
    window of each core may be short)."""
    import heapq
    deg = np.bincount(dst, minlength=n_nodes).astype(np.int64)
    nc_nodes = n_nodes // ncores
    nwin = (nc_nodes + win - 1) // win
    base = []
    cap = []
    for c in range(ncores):
        for w in range(nwin):
            base.append(c * nc_nodes + w * win)
            cap.append(min(win, nc_nodes - w * win))
    nbins = len(base)
    order = np.argsort(-deg, kind="stable")
    heap = [(0, b) for b in range(nbins)]
    heapq.heapify(heap)
    slot = [0] * nbins
    perm = np.zeros(n_nodes, dtype=np.int64)
    for node in order:
        while True:
            load, b = heapq.heappop(heap)
            if slot[b] < cap[b]:
                break
        perm[node] = base[b] + slot[b]
        slot[b] += 1
        if slot[b] < cap[b]:
            heapq.heappush(heap, (load + deg[node], b))
    return perm


def chunk_layout(n_nodes, ncores, chunk_tiles):
    """Chunked-AllGather table layout. Returns (bounds, rowmap) where bounds
    are per-core local row boundaries of each chunk and rowmap[node] is the
    table row of a global node id under chunk-major ordering."""
    nc_nodes = n_nodes // ncores
    bounds = []
    lo = 0
    while lo < nc_nodes:
        hi = min(lo + chunk_tiles * P, nc_nodes)
        bounds.append((lo, hi))
        lo = hi
    rowmap = np.zeros(n_nodes, dtype=np.int64)
    out_base = 0
    for (lo, hi) in bounds:
        s = hi - lo
        for c in range(ncores):
            nodes = np.arange(c * nc_nodes + lo, c * nc_nodes + hi)
            rowmap[nodes] = out_base + c * s + np.arange(s)
        out_base += ncores * s
    return bounds, rowmap


def build_host_inputs(x, edge_index, batch, W1, att_src1, att_dst1, b1,
                      W2, att_src2, att_dst2, b2, Wfc, bfc,
                      n_nodes, n_graphs, ncores, win):
    src, dst = np.asarray(edge_index[0]), np.asarray(edge_index[1])
    nc_nodes = n_nodes // ncores
    nt = (nc_nodes + P - 1) // P
    ncpad = nt * P

    bounds, rowmap = chunk_layout(n_nodes, ncores, CHUNK_TILES)
    tlo, thi, t0, edata = build_edge_data(
        rowmap[src.astype(np.int64)], dst.astype(np.int64), n_nodes, ncores, win)

    # augmented weights: a = x @ (W @ att) computed in the same matmul as h
    in_c = W1.shape[0]
    A1 = np.zeros((in_c, 2 * HEADS), dtype=np.float64)
    for h in range(HEADS):
        A1[:, h] = W1[:, h * HID_C:(h + 1) * HID_C].astype(np.float64) @ att_src1[h].astype(np.float64)
        A1[:, HEADS + h] = W1[:, h * HID_C:(h + 1) * HID_C].astype(np.float64) @ att_dst1[h].astype(np.float64)
    W1aug = np.concatenate([W1.astype(np.float64), A1], axis=1).astype(BF16)  # [in_c, 264]

    hid2 = W2.shape[0]
    A2 = np.zeros((hid2, 2), dtype=np.float64)
    A2[:, 0] = W2.astype(np.float64) @ att_src2[0].astype(np.float64)
    A2[:, 1] = W2.astype(np.float64) @ att_dst2[0].astype(np.float64)
    W2aug = np.concatenate([W2.astype(np.float64), A2], axis=1).astype(BF16)  # [hid2, 258]
    W2pad = np.zeros((hid2, TBLC), dtype=BF16)
    W2pad[:, :258] = W2aug

    # graph-mean map and counts
    cnt = np.bincount(batch, minlength=n_graphs).astype(np.float32)
    cnt_inv = (1.0 / np.maximum(cnt, 1.0)).astype(np.float32)

    out_c = Wfc.shape[0]
    common = dict(
        w1aug=np.ascontiguousarray(W1aug),
        w2aug=np.ascontiguousarray(W2pad),
        wfc=np.ascontiguousarray(Wfc.astype(BF16)),
        b1rep=np.ascontiguousarray(np.broadcast_to(b1.astype(np.float32), (win, b1.shape[0])).copy()),
        b2rep=np.ascontiguousarray(np.broadcast_to(b2.astype(np.float32), (win, b2.shape[0])).copy()),
        bfc2=np.ascontiguousarray(bfc.astype(np.float32).reshape(2, P).T.copy()),
        cinv=np.ascontiguousarray(np.broadcast_to(cnt_inv, (P, n_graphs)).copy()),
    )

    per_core = []
    for c in range(ncores):
        xt = np.zeros((in_c, ncpad), dtype=BF16)
        xs = x[c * nc_nodes:(c + 1) * nc_nodes].astype(np.float32)
        xt[:, :nc_nodes] = np.ascontiguousarray(xs.T).astype(BF16)
        gmap = np.zeros((nt, P, n_graphs), dtype=np.float32)
        nodes = np.arange(nc_nodes)
        gmap[nodes // P, nodes % P, batch[c * nc_nodes:(c + 1) * nc_nodes]] = 1.0
        d = edata[c]
        per_core.append(dict(
            xt=xt,
            idx16=np.ascontiguousarray(d["idx16"]),
            oh_e=np.ascontiguousarray(d["oh_e"]),
            oh_d=np.ascontiguousarray(d["oh_d"]),
            gmap=np.ascontiguousarray(gmap.astype(BF16)),
            **common,
        ))
    return tlo, thi, t0, per_core


# --------------------------------------------------------------------------
# device program
# --------------------------------------------------------------------------

def build_program(tlo, thi, t0, n_nodes, n_graphs, ncores, win,
                  dma_scratch=16384):
    bounds, _ = chunk_layout(n_nodes, ncores, CHUNK_TILES)
    from concourse import bass, bacc, mybir, tile
    from concourse.masks import make_identity
    from concourse.library_config import mlp

    DT = mybir.dt.bfloat16
    F32 = mybir.dt.float32
    F8 = mybir.dt.float8e4
    U8 = mybir.dt.uint8
    AF = mybir.ActivationFunctionType
    OP = mybir.AluOpType

    nc_nodes = n_nodes // ncores
    nt = (nc_nodes + P - 1) // P
    nwin = (nc_nodes + win - 1) // win
    ntw = tlo + thi
    ttot = int(ntw.sum())
    twmax = int(ntw.max())
    in_c, out_c = IN_C, OUT_C
    G = n_graphs

    nc = bacc.Bacc("TRN2", target_bir_lowering=False, num_devices=ncores,
                   dynamic_dma_scratch_size=dma_scratch, num_swdge_queues=4)

    # ---- dram i/o ----
    xt_d = nc.dram_tensor("xt", [in_c, nt * P], DT, kind="ExternalInput")
    w1_d = nc.dram_tensor("w1aug", [in_c, TBLC], DT, kind="ExternalInput")
    w2_d = nc.dram_tensor("w2aug", [in_c, TBLC], DT, kind="ExternalInput")
    wfc_d = nc.dram_tensor("wfc", [out_c, out_c], DT, kind="ExternalInput")
    idx_d = nc.dram_tensor("idx16", [P, ttot * 8], mybir.dt.int16, kind="ExternalInput")
    ohe_d = nc.dram_tensor("oh_e", [ttot, P, win], F8, kind="ExternalInput")
    ohd_d = nc.dram_tensor("oh_d", [ttot, win, P], F8, kind="ExternalInput")
    gmap_d = nc.dram_tensor("gmap", [nt, P, G], DT, kind="ExternalInput")
    b1_d = nc.dram_tensor("b1rep", [win, out_c], F32, kind="ExternalInput")
    b2_d = nc.dram_tensor("b2rep", [win, out_c], F32, kind="ExternalInput")
    bfc_d = nc.dram_tensor("bfc2", [P, 2], F32, kind="ExternalInput")
    cinv_d = nc.dram_tensor("cinv", [P, G], F32, kind="ExternalInput")
    y_d = nc.dram_tensor("y", [out_c, G], F32, kind="ExternalOutput")

    UB = 274  # useful row bytes: 257 h+one, pad, a-vals at 258:274
    cin1 = nc.dram_tensor("cin1", [nc_nodes, UB], U8, kind="Internal")
    tab1c = nc.dram_tensor("tab1c", [n_nodes, UB], U8, kind="Internal", addr_space="Shared")
    tab1 = nc.dram_tensor("tab1", [n_nodes, ROWB], U8, kind="Internal")
    cin2 = nc.dram_tensor("cin2", [nc_nodes, UB], U8, kind="Internal")
    tab2c = nc.dram_tensor("tab2c", [n_nodes, UB], U8, kind="Internal", addr_space="Shared")
    tab2 = nc.dram_tensor("tab2", [n_nodes, ROWB], U8, kind="Internal")
    pin = nc.dram_tensor("pin", [out_c, G], F32, kind="Internal")
    pout = nc.dram_tensor("pout", [out_c, G], F32, kind="Internal", addr_space="Shared")

    groups = [list(range(ncores))]

    with tile.TileContext(nc) as tc:
        with (
            tc.tile_pool(name="const", bufs=1) as cpool,
            tc.tile_pool(name="work", bufs=3) as wpool,
            tc.tile_pool(name="gath", bufs=4) as gpool,
            tc.tile_pool(name="rhsp", bufs=3) as rpool,
            tc.tile_pool(name="np", bufs=2, space="PSUM") as npp,
            tc.tile_pool(name="agg", bufs=2, space="PSUM") as aggp,
            tc.tile_pool(name="adp", bufs=1, space="PSUM") as adp,
            tc.tile_pool(name="trp", bufs=1, space="PSUM") as trp,
            tc.tile_pool(name="plp", bufs=1, space="PSUM") as plp,
        ):
            nc.gpsimd.load_library(mlp)
            # ---- constants ----
            ident = cpool.tile([P, P], DT)
            make_identity(nc, ident[:])
            w1_sb = cpool.tile([P, 2, TBLC], DT)
            nc.sync.dma_start(out=w1_sb[:, :, :], in_=w1_d.ap().rearrange("(kh p) m -> p kh m", p=P))
            w2_sb = cpool.tile([P, 2, TBLC], DT)
            nc.sync.dma_start(out=w2_sb[:, :, :], in_=w2_d.ap().rearrange("(kh p) m -> p kh m", p=P))
            wfc_sb = cpool.tile([P, 2, 2, P], DT)  # [k-half, m-half]
            nc.sync.dma_start(out=wfc_sb[:, :, :, :],
                              in_=wfc_d.ap().rearrange("(kh p) (mh q) -> p kh mh q", p=P, q=P))
            b1_sb = cpool.tile([win, out_c], F32)
            nc.sync.dma_start(out=b1_sb[:, :], in_=b1_d[:, :])
            b2_sb = cpool.tile([win, out_c], F32)
            nc.sync.dma_start(out=b2_sb[:, :], in_=b2_d[:, :])
            bfc_sb = cpool.tile([P, 2], F32)
            nc.sync.dma_start(out=bfc_sb[:, :], in_=bfc_d[:, :])
            cinv_sb = cpool.tile([P, G], F32)
            nc.sync.dma_start(out=cinv_sb[:, :], in_=cinv_d[:, :])
            isb = cpool.tile([P, ttot * 8], mybir.dt.int16)
            nc.sync.dma_start(out=isb[:, :], in_=idx_d[:, :])

            # ---- node phase ----
            # chunk bookkeeping for interleaved AllGathers
            chunk_end_tile = {}
            chunk_ob = []
            ob = 0
            for k, (lo, hi) in enumerate(bounds):
                chunk_end_tile[(hi + P - 1) // P - 1] = k
                chunk_ob.append(ob)
                ob += ncores * (hi - lo)

            def ag_chunk(cin, tabc, tab, k):
                # compact rows over the links, then a local restride DMA into
                # the 512B-strided gather table
                lo, hi = bounds[k]
                s = hi - lo
                g0, g1 = chunk_ob[k], chunk_ob[k] + ncores * s
                nc.gpsimd.collective_compute(
                    "AllGather", mybir.AluOpType.bypass,
                    ins=[cin.ap()[lo:hi, :]],
                    outs=[tabc.ap()[g0:g1, :]],
                    replica_groups=groups)
                nc.sync.dma_start(out=tab.ap()[g0:g1, 0:UB], in_=tabc.ap()[g0:g1, :])

            def node_tile(t, lhsT_of, w_sb, ocols, cin):
                rows = min(P, nc_nodes - t * P)
                h2 = ocols - 256  # a-val cols (2*H)
                ps = npp.tile([P, TBLC], F32, tag="nps", name="nps")
                for kh in range(2):
                    nc.tensor.matmul(out=ps[:rows, :ocols], lhsT=lhsT_of(t, kh, rows),
                                     rhs=w_sb[:, kh, :ocols], start=(kh == 0), stop=(kh == 1))
                # row bytes: [h fp8 0:256 | one fp8 256 | pad | a_src bf16 258: | a_dst]
                hf8 = wpool.tile([P, 257], F8, tag="hf8", name="hf8")
                nc.vector.tensor_copy(out=hf8[:rows, 0:256], in_=ps[:rows, 0:256])
                nc.vector.memset(hf8[:rows, 256:257], 1.0)
                av = wpool.tile([P, 8], DT, tag="av", name="av")
                nc.vector.tensor_copy(out=av[:rows, :h2], in_=ps[:rows, 256:256 + h2])
                nc.sync.dma_start(out=cin.ap()[t * P:t * P + rows, 0:257],
                                  in_=hf8[:rows, :].bitcast(U8))
                nc.sync.dma_start(out=cin.ap()[t * P:t * P + rows, 258:258 + 2 * h2],
                                  in_=av[:rows, :h2].bitcast(U8))

            def node_phase(lhsT_of, w_sb, ocols, cin, tabc=None, tab=None):
                for t in range(nt):
                    node_tile(t, lhsT_of, w_sb, ocols, cin)
                    if tab is not None and t in chunk_end_tile:
                        ag_chunk(cin, tabc, tab, chunk_end_tile[t])

            # layer-1 node phase: xT staged in two bulk DMAs, sliced per tile
            xt_sb = cpool.tile([P, 2, nt * P], DT)
            xstep = ((nt + 4) // 5) * P
            for lo in range(0, nt * P, xstep):
                hi = min(lo + xstep, nt * P)
                for kh in range(2):
                    nc.sync.dma_start(out=xt_sb[:, kh, lo:hi], in_=xt_d[kh * P:(kh + 1) * P, lo:hi])
            def l1_lhsT(t, kh, rows):
                return xt_sb[:, kh, t * P:t * P + rows]
            node_phase(l1_lhsT, w1_sb, 256 + 2 * HEADS, cin1, tabc=tab1c, tab=tab1)

            # ---- edge phase ----
            def edge_phase(tab, cin, brep, H, pool_into=None, after_window=None,
                           direct=False):
                """pool_into: optional psum pair — accumulate graph-pool matmuls
                from the relu output instead of writing it to DRAM.
                direct (H==1): exp-scaled onehot lhsT x raw fp8 gathered rhs;
                the ones byte at row offset 256 yields the softmax denominator
                in the same matmul (out col 256)."""
                RH = (out_c + 1) if direct else (H + out_c)
                ab0 = 258            # a_src byte offset in table row
                db0 = 258 + 2 * H    # a_dst byte offset
                for w in range(nwin):
                    size = min(win, nc_nodes - w * win)
                    lo_t, hi_t, base = int(tlo[w]), int(thi[w]), int(t0[w])
                    tw = lo_t + hi_t
                    s0 = base * P
                    # own-dst attention coefs
                    ad = wpool.tile([win, HEADS], DT, tag="adst")
                    nc.sync.dma_start(out=ad[:size, :H],
                                      in_=cin.ap()[w * win:w * win + size, db0:db0 + 2 * H].bitcast(DT))
                    # onehots for this window (fp8: 0/1 exact)
                    ohe = wpool.tile([P, twmax, win], F8, tag="ohe")
                    nc.sync.dma_start(out=ohe[:, :tw, :],
                                      in_=ohe_d.ap()[base:base + tw, :, :].rearrange("t p d -> p t d"))
                    ohd = wpool.tile([win, twmax, P], F8, tag="ohd")
                    nc.sync.dma_start(out=ohd[:, :tw, :],
                                      in_=ohd_d.ap()[base:base + tw, :, :].rearrange("t d e -> d t e"))
                    # gather source rows: batched SWDGE gather per table half,
                    # chunked to <=1024 indices per call (HW ucode limit)
                    g = gpool.tile([P, twmax, ROWB], U8, tag="g")
                    for tb, tn, ta in ((0, lo_t, tab.ap()[0:SPLIT, :]),
                                       (lo_t, hi_t, tab.ap()[SPLIT:n_nodes, :])):
                        for c0 in range(0, tn, 8):
                            cn = min(8, tn - c0)
                            K = cn * P
                            sa = s0 + (tb + c0) * P
                            nc.gpsimd.dma_gather(
                                g[:, tb + c0:tb + c0 + cn, :], ta,
                                isb[:, sa // 16:(sa + K) // 16], K, K, ROWB)
                    # a_dst expanded to edges: [128e, tw*H]
                    adps = adp.tile([P, twmax * HEADS], F32, tag="adps")
                    for t in range(tw):
                        nc.tensor.matmul(out=adps[:, t * H:(t + 1) * H],
                                         lhsT=ohd[:size, t, :], rhs=ad[:size, :H],
                                         start=True, stop=True)
                    # logits -> exp(leaky_relu)
                    lg = wpool.tile([P, twmax * HEADS], F32, tag="lg")
                    lg3 = lg[:, :tw * H].rearrange("p (t h) -> p t h", t=tw)
                    ad3 = adps[:, :tw * H].rearrange("p (t h) -> p t h", t=tw)
                    nc.vector.tensor_tensor(out=lg3, in0=g[:, :tw, ab0:ab0 + 2 * H].bitcast(DT),
                                            in1=ad3, op=OP.add)
                    lk = wpool.tile([P, twmax * HEADS], F32, tag="lk")
                    nc.vector.scalar_tensor_tensor(out=lk[:, :tw * H], in0=lg[:, :tw * H],
                                                   scalar=NEG_SLOPE, in1=lg[:, :tw * H],
                                                   op0=OP.mult, op1=OP.max)
                    ag = aggp.tile([win, RH], F32, tag="ag")
                    if direct:
                        # exp-scaled onehot; rhs is raw gathered fp8 [h | one]
                        et = wpool.tile([P, twmax], DT, tag="et")
                        nc.scalar.activation(out=et[:, :tw], in_=lk[:, :tw], func=AF.Exp)
                        ohs = rpool.tile([P, twmax, win], F8, tag="ohs")
                        nc.vector.tensor_tensor(out=ohs[:, :tw, :], in0=ohe[:, :tw, :],
                                                in1=et[:, :tw].to_broadcast([P, tw, win]),
                                                op=OP.mult)
                        for t in range(tw):
                            nc.tensor.matmul(out=ag[:, :], lhsT=ohs[:, t, :],
                                             rhs=g[:, t, 0:RH].bitcast(F8),
                                             start=(t == 0), stop=(t == tw - 1))
                    else:
                        rhs = rpool.tile([P, twmax, RH], DT, tag="rhs")
                        nc.scalar.activation(out=rhs[:, :tw, 0:H],
                                             in_=lk[:, :tw * H].rearrange("p (t h) -> p t h", t=tw),
                                             func=AF.Exp)
                        # rhs features = exp * h_src, one clean 3-d op per head
                        for h in range(H):
                            nc.vector.tensor_tensor(
                                out=rhs[:, :tw, H + h * 64:H + (h + 1) * 64],
                                in0=g[:, :tw, h * 64:(h + 1) * 64].bitcast(F8),
                                in1=rhs[:, :tw, h:h + 1].to_broadcast([P, tw, 64]),
                                op=OP.mult)
                        for t in range(tw):
                            nc.tensor.matmul(out=ag[:, :], lhsT=ohe[:, t, :], rhs=rhs[:, t, :],
                                             start=(t == 0), stop=(t == tw - 1))
                    # normalize + bias + relu
                    dcol = out_c if direct else 0  # denominator column base in ag
                    fcol = 0 if direct else H      # feature column base
                    s = wpool.tile([win, HEADS], F32, tag="s")
                    nc.vector.tensor_scalar_max(s[:size, :H], ag[:size, dcol:dcol + H], 1e-30)
                    nc.vector.reciprocal(out=s[:size, :H], in_=s[:size, :H])
                    on = wpool.tile([win, out_c], F32, tag="on")
                    nc.vector.tensor_tensor(
                        out=on[:size, :].rearrange("d (h c) -> d h c", h=H),
                        in0=ag[:size, fcol:fcol + out_c].rearrange("d (h c) -> d h c", h=H),
                        in1=s[:size, :H].to_broadcast([size, H, out_c // H]), op=OP.mult)
                    nc.vector.tensor_tensor(out=on[:size, :], in0=on[:size, :],
                                            in1=brep[:size, :], op=OP.add)
                    ro = wpool.tile([win, out_c], DT, tag="ro")
                    nc.scalar.activation(out=ro[:size, :], in_=on[:size, :], func=AF.Relu)
                    if after_window is not None:
                        after_window(w, ro, size)
                    if pool_into is not None:
                        gm = wpool.tile([P, G], DT, tag="gm")
                        nc.sync.dma_start(out=gm[:, :], in_=gmap_d[w, :, :])
                        for mh in range(2):
                            nc.tensor.matmul(out=pool_into[mh][:, :],
                                             lhsT=ro[:size, mh * P:(mh + 1) * P],
                                             rhs=gm[:size, :],
                                             start=(w == 0), stop=(w == nwin - 1))

            def l1_after_window(w, ro, size):
                def l2_lhsT(t, kh, rows):
                    tp = trp.tile([P, P], DT, tag="tp", name="tp")
                    nc.tensor.transpose(out=tp[:, :rows], in_=ro[:rows, kh * P:(kh + 1) * P],
                                        identity=ident[:rows, :rows])
                    tl = wpool.tile([P, P], DT, tag="tl", name="tl")
                    nc.vector.tensor_copy(out=tl[:, :rows], in_=tp[:, :rows])
                    return tl[:, :rows]
                node_tile(w, l2_lhsT, w2_sb, 258, cin2)
                if w in chunk_end_tile:
                    ag_chunk(cin2, tab2c, tab2, chunk_end_tile[w])

            edge_phase(tab1, cin1, b1_sb, HEADS, after_window=l1_after_window)

            assert win == P and nwin == nt
            pps = [plp.tile([P, G], F32, tag=f"pp{mh}", name=f"pp{mh}") for mh in range(2)]
            edge_phase(tab2, cin2, b2_sb, 1, pool_into=pps, direct=True)

            # ---- pool + fc ----
            psb = wpool.tile([P, 2, G], F32, tag="psb")
            for mh in range(2):
                nc.vector.tensor_copy(out=psb[:, mh, :], in_=pps[mh][:, :])
            nc.sync.dma_start(out=pin.ap().rearrange("(mh p) g -> p mh g", p=P), in_=psb[:, :, :])

            nc.gpsimd.collective_compute(
                "AllReduce", mybir.AluOpType.add,
                ins=[pin.ap()], outs=[pout.ap()], replica_groups=groups)

            pr = wpool.tile([P, 2, G], F32, tag="pr")
            nc.sync.dma_start(out=pr[:, :, :], in_=pout.ap().rearrange("(mh p) g -> p mh g", p=P))
            pm = wpool.tile([P, 2, G], DT, tag="pm")
            for kh in range(2):
                nc.vector.tensor_tensor(out=pm[:, kh, :], in0=pr[:, kh, :], in1=cinv_sb[:, :], op=OP.mult)
            for mh in range(2):
                fps = aggp.tile([P, G], F32, tag="ag")
                for kh in range(2):
                    nc.tensor.matmul(out=fps[:, :], lhsT=wfc_sb[:, kh, mh, :], rhs=pm[:, kh, :],
                                     start=(kh == 0), stop=(kh == 1))
                yo = wpool.tile([P, G], F32, tag="yo")
                nc.scalar.activation(out=yo[:, :], in_=fps[:, :], func=AF.Relu,
                                     bias=bfc_sb[:, mh:mh + 1], scale=1.0)
                nc.sync.dma_start(out=y_d[mh * P:(mh + 1) * P, :], in_=yo[:, :])

    # Spread gathers over the 4 SWDGE queues for parallel descriptor
    # generation. Tile sem assignment rotates the 8 DMASW lanes over Pool DMA
    # instructions in scheduled (block) order; queue = lane % 4 keeps each
    # lane pinned to one queue.
    k = 0
    for b in nc.main_func.blocks:
        for i in b.instructions:
            if isinstance(i, mybir.InstDMAGatherAnt):
                i.queue_num = (k % 8) % 4
                k += 1

    nc.compile()
    return nc


def _install_ntff_hook():
    """Register the NTFF profile hook (the image's antenv lacks axon_hooks)."""
    import types
    mod = sys.modules.get("antenv.axon_hooks")
    if mod is None:
        import antenv
        mod = types.ModuleType("antenv.axon_hooks")
        mod._hook = None
        mod.set_axon_ntff_profile_hook = lambda h: setattr(mod, "_hook", h)
        mod.get_axon_ntff_profile_hook = lambda: mod._hook
        sys.modules["antenv.axon_hooks"] = mod
        antenv.axon_hooks = mod
    if mod._hook is None:
        from trn_agent_boot.trn_boot import _ntff_profile_via_ctypes
        mod.set_axon_ntff_profile_hook(_ntff_profile_via_ctypes("/opt/axon/libaxon_pjrt.so"))

# --------------------------------------------------------------------------
# entry point
# --------------------------------------------------------------------------

def kernel(**inputs) -> np.ndarray:
    global LAST_EXEC_NS
    from concourse.bass_utils import run_bass_kernel_spmd

    args = {k: np.asarray(v) for k, v in inputs.items()}
    perm = balance_nodes(args["edge_index"][1], N_NODES, NCORES, WIN)
    old_of_new = np.argsort(perm)
    args["x"] = args["x"][old_of_new]
    args["batch"] = args["batch"][old_of_new]
    ei = args["edge_index"]
    args["edge_index"] = np.stack([perm[ei[0]], perm[ei[1]]]).astype(ei.dtype)
    tlo, thi, t0, per_core = build_host_inputs(
        args["x"], args["edge_index"], args["batch"],
        args["W1"], args["att_src1"], args["att_dst1"], args["b1"],
        args["W2"], args["att_src2"], args["att_dst2"], args["b2"],
        args["Wfc"], args["bfc"],
        N_NODES, N_GRAPHS, NCORES, WIN)
    nc = build_program(tlo, thi, t0, N_NODES, N_GRAPHS, NCORES, WIN)

    trace = os.environ.get("GAT_TRACE") == "1"
    if trace:
        try:
            _install_ntff_hook()
        except Exception:
            trace = False
    res = run_bass_kernel_spmd(nc, per_core, core_ids=list(range(NCORES)), trace=trace)
    LAST_EXEC_NS = res.exec_time_ns
    y = res.results[0]["y"]
    return np.ascontiguousarray(y.T).astype(np.float32)


# revision 38
# speedup vs baseline: 1.2270x; 1.2270x over previous
"""GAT (2-layer, PyG-style) on 8 Trainium2 NeuronCores.

Strategy (matches the node/graph-parallel sharding hint):
  - Nodes partitioned into 8 contiguous ranges (6250/core); edges assigned to
    the core owning their DST node, sorted by dst, grouped per 128-dst window
    and padded to 128-edge tiles.
  - Node phase sharded: each core computes h_aug = x @ [W | W@att] for its own
    nodes; a chunked AllGather replicates the gather table (512-byte rows:
    256 fp8 features + 2H bf16 attention coefs + pad).
  - Edge phase per core: per-edge source rows fetched with the batched SWDGE
    dma_gather ucode (int16 indices, 16-partition wrapped); the table is split
    in two halves at the chunk boundary so indices fit int16. Attention
    logits assembled on-chip; segment softmax + scatter-add done as TensorE
    matmuls against host-built 0/1 matrices (onehot per (edge, dst-window)),
    with exp folded into the matmul rhs and 1/sum applied per-dst at the end.
  - Global mean-pool via matmul with a host-built node->graph 0/1 map,
    AllReduce of the [256, 50] partial, then the FC layer replicated.
"""

import os
import sys

sys.path.insert(0, "/opt/trn_rl_repo")

import numpy as np
import ml_dtypes

N_NODES, N_EDGES = 50000, 800000
IN_C, HID_C, OUT_C, HEADS = 256, 64, 256, 4
N_GRAPHS = 50
NEG_SLOPE = 0.2
NCORES = 8
WIN = 128         # dst nodes per aggregation window (psum partitions)
ROWB = 512        # gather-table row bytes: 256 fp8 feat + 2H bf16 a + pad
TBLC = 264        # max node-phase matmul output cols (256 h + 2H a-vals)
P = 128
CHUNK_TILES = 13  # node-tiles per AllGather chunk (4 chunks of 6250 rows)
SPLIT = NCORES * 2 * CHUNK_TILES * P  # 26624: table halves = 2 chunks each

BF16 = ml_dtypes.bfloat16

LAST_EXEC_NS = None  # set by kernel() when GAT_TRACE=1


# --------------------------------------------------------------------------
# host-side preprocessing
# --------------------------------------------------------------------------

def build_edge_data(src_rows, dst, n_nodes, ncores, win):
    """Per-core int16 gather indices + onehot matrices.

    Edges are ordered by (core, dst-window, lo/hi table half); each
    (window, half) run is padded to 128-edge tiles with a tile count shared
    across cores. Returns (tlo, thi, t0, percore) where percore[c] has
    idx16 [128, ttot*8] (16-partition-wrapped, replicated), oh_e/oh_d.
    """
    import ml_dtypes as _md
    nc_nodes = n_nodes // ncores
    nwin = (nc_nodes + win - 1) // win
    core_of = dst // nc_nodes
    win_of = (dst % nc_nodes) // win
    grp_of = (src_rows >= SPLIT).astype(np.int64)
    gid = (core_of * nwin + win_of) * 2 + grp_of
    order = np.argsort(gid, kind="stable")
    s_rows, s_dst = src_rows[order], dst[order]
    core_of, win_of, grp_of, gid = (a[order] for a in (core_of, win_of, grp_of, gid))

    counts = np.bincount(gid, minlength=ncores * nwin * 2).reshape(ncores, nwin, 2)
    twg = ((counts + P - 1) // P).max(axis=0)  # [nwin, 2] tiles, shared
    for w in range(nwin):
        if twg[w].sum() == 0:
            twg[w, 0] = 1
    tlo, thi = twg[:, 0].astype(np.int64), twg[:, 1].astype(np.int64)
    ntw = tlo + thi
    t0 = np.concatenate([[0], np.cumsum(ntw)])[:-1]
    ttot = int(ntw.sum())

    E = gid.shape[0]
    starts = np.concatenate([[0], np.cumsum(counts.ravel())])[:-1]
    k_in_g = np.arange(E) - starts[gid]
    tile_base = np.zeros((nwin, 2), dtype=np.int64)
    tile_base[:, 0] = t0
    tile_base[:, 1] = t0 + tlo
    gtile_all = tile_base[win_of, grp_of] + k_in_g // P
    lane = (k_in_g % P).astype(np.int64)
    dloc = (s_dst - (core_of * nc_nodes + win_of * win)).astype(np.int64)
    val = (s_rows - grp_of * SPLIT).astype(np.int16)

    percore = []
    for c in range(ncores):
        m = core_of == c
        slots = gtile_all[m] * P + lane[m]
        idx16 = np.zeros((16, ttot * 8), dtype=np.int16)
        idx16[slots % 16, slots // 16] = val[m]
        oh_e = np.zeros((ttot, P, win), dtype=np.float32)
        oh_d = np.zeros((ttot, win, P), dtype=np.float32)
        oh_e[gtile_all[m], lane[m], dloc[m]] = 1.0
        oh_d[gtile_all[m], dloc[m], lane[m]] = 1.0
        percore.append(dict(idx16=np.tile(idx16, (8, 1)),
                            oh_e=oh_e.astype(_md.float8_e4m3),
                            oh_d=oh_d.astype(_md.float8_e4m3)))
    return tlo, thi, t0, percore


def balance_nodes(dst, n_nodes, ncores, win):
    """Relabel nodes so each (core, window) bin carries a near-equal edge
    count: perm[old_id] = new_id. Greedy largest-degree-first into the
    lightest non-full bin. Bins follow the per-core window layout (the last
    window of each core may be short)."""
    import heapq
    deg = np.bincount(dst, minlength=n_nodes).astype(np.int64)
    nc_nodes = n_nodes // ncores
    nwin = (nc_nodes + win - 1) // win
    base = []
    cap = []
    for c in range(ncores):
        for w in range(nwin):
            base.append(c * nc_nodes + w * win)
            cap.append(min(win, nc_nodes - w * win))
    nbins = len(base)
    order = np.argsort(-deg, kind="stable")
    heap = [(0, b) for b in range(nbins)]
    heapq.heapify(heap)
    slot = [0] * nbins
    perm = np.zeros(n_nodes, dtype=np.int64)
    for node in order:
        while True:
            load, b = heapq.heappop(heap)
            if slot[b] < cap[b]:
                break
        perm[node] = base[b] + slot[b]
        slot[b] += 1
        if slot[b] < cap[b]:
            heapq.heappush(heap, (load + deg[node], b))
    return perm


def chunk_layout(n_nodes, ncores, chunk_tiles):
    """Chunked-AllGather table layout. Returns (bounds, rowmap) where bounds
    are per-core local row boundaries of each chunk and rowmap[node] is the
    table row of a global node id under chunk-major ordering."""
    nc_nodes = n_nodes // ncores
    bounds = []
    lo = 0
    while lo < nc_nodes:
        hi = min(lo + chunk_tiles * P, nc_nodes)
        bounds.append((lo, hi))
        lo = hi
    rowmap = np.zeros(n_nodes, dtype=np.int64)
    out_base = 0
    for (lo, hi) in bounds:
        s = hi - lo
        for c in range(ncores):
            nodes = np.arange(c * nc_nodes + lo, c * nc_nodes + hi)
            rowmap[nodes] = out_base + c * s + np.arange(s)
        out_base += ncores * s
    return bounds, rowmap


def build_host_inputs(x, edge_index, batch, W1, att_src1, att_dst1, b1,
                      W2, att_src2, att_dst2, b2, Wfc, bfc,
                      n_nodes, n_graphs, ncores, win):
    src, dst = np.asarray(edge_index[0]), np.asarray(edge_index[1])
    nc_nodes = n_nodes // ncores
    nt = (nc_nodes + P - 1) // P
    ncpad = nt * P

    bounds, rowmap = chunk_layout(n_nodes, ncores, CHUNK_TILES)
    tlo, thi, t0, edata = build_edge_data(
        rowmap[src.astype(np.int64)], dst.astype(np.int64), n_nodes, ncores, win)

    # augmented weights: a = x @ (W @ att) computed in the same matmul as h
    in_c = W1.shape[0]
    A1 = np.zeros((in_c, 2 * HEADS), dtype=np.float64)
    for h in range(HEADS):
        A1[:, h] = W1[:, h * HID_C:(h + 1) * HID_C].astype(np.float64) @ att_src1[h].astype(np.float64)
        A1[:, HEADS + h] = W1[:, h * HID_C:(h + 1) * HID_C].astype(np.float64) @ att_dst1[h].astype(np.float64)
    W1aug = np.concatenate([W1.astype(np.float64), A1], axis=1).astype(BF16)  # [in_c, 264]

    hid2 = W2.shape[0]
    A2 = np.zeros((hid2, 2), dtype=np.float64)
    A2[:, 0] = W2.astype(np.float64) @ att_src2[0].astype(np.float64)
    A2[:, 1] = W2.astype(np.float64) @ att_dst2[0].astype(np.float64)
    W2aug = np.concatenate([W2.astype(np.float64), A2], axis=1).astype(BF16)  # [hid2, 258]
    W2pad = np.zeros((hid2, TBLC), dtype=BF16)
    W2pad[:, :258] = W2aug

    # graph-mean map and counts
    cnt = np.bincount(batch, minlength=n_graphs).astype(np.float32)
    cnt_inv = (1.0 / np.maximum(cnt, 1.0)).astype(np.float32)

    out_c = Wfc.shape[0]
    common = dict(
        w1aug=np.ascontiguousarray(W1aug),
        w2aug=np.ascontiguousarray(W2pad),
        wfc=np.ascontiguousarray(Wfc.astype(BF16)),
        b1rep=np.ascontiguousarray(np.broadcast_to(b1.astype(np.float32), (win, b1.shape[0])).copy()),
        b2rep=np.ascontiguousarray(np.broadcast_to(b2.astype(np.float32), (win, b2.shape[0])).copy()),
        bfc2=np.ascontiguousarray(bfc.astype(np.float32).reshape(2, P).T.copy()),
        cinv=np.ascontiguousarray(np.broadcast_to(cnt_inv, (P, n_graphs)).copy()),
    )

    per_core = []
    for c in range(ncores):
        xt = np.zeros((in_c, ncpad), dtype=BF16)
        xs = x[c * nc_nodes:(c + 1) * nc_nodes].astype(np.float32)
        xt[:, :nc_nodes] = np.ascontiguousarray(xs.T).astype(BF16)
        gmap = np.zeros((nt, P, n_graphs), dtype=np.float32)
        nodes = np.arange(nc_nodes)
        gmap[nodes // P, nodes % P, batch[c * nc_nodes:(c + 1) * nc_nodes]] = 1.0
        d = edata[c]
        per_core.append(dict(
            xt=xt,
            idx16=np.ascontiguousarray(d["idx16"]),
            oh_e=np.ascontiguousarray(d["oh_e"]),
            oh_d=np.ascontiguousarray(d["oh_d"]),
            gmap=np.ascontiguousarray(gmap.astype(BF16)),
            **common,
        ))
    return tlo, thi, t0, per_core


# --------------------------------------------------------------------------
# device program
# --------------------------------------------------------------------------

def build_program(tlo, thi, t0, n_nodes, n_graphs, ncores, win,
                  dma_scratch=16384):
    bounds, _ = chunk_layout(n_nodes, ncores, CHUNK_TILES)
    from concourse import bass, bacc, mybir, tile
    from concourse.masks import make_identity
    from concourse.library_config import mlp

    DT = mybir.dt.bfloat16
    F32 = mybir.dt.float32
    F8 = mybir.dt.float8e4
    U8 = mybir.dt.uint8
    AF = mybir.ActivationFunctionType
    OP = mybir.AluOpType

    nc_nodes = n_nodes // ncores
    nt = (nc_nodes + P - 1) // P
    nwin = (nc_nodes + win - 1) // win
    ntw = tlo + thi
    ttot = int(ntw.sum())
    twmax = int(ntw.max())
    in_c, out_c = IN_C, OUT_C
    G = n_graphs

    nc = bacc.Bacc("TRN2", target_bir_lowering=False, num_devices=ncores,
                   dynamic_dma_scratch_size=dma_scratch, num_swdge_queues=4)

    # ---- dram i/o ----
    xt_d = nc.dram_tensor("xt", [in_c, nt * P], DT, kind="ExternalInput")
    w1_d = nc.dram_tensor("w1aug", [in_c, TBLC], DT, kind="ExternalInput")
    w2_d = nc.dram_tensor("w2aug", [in_c, TBLC], DT, kind="ExternalInput")
    wfc_d = nc.dram_tensor("wfc", [out_c, out_c], DT, kind="ExternalInput")
    idx_d = nc.dram_tensor("idx16", [P, ttot * 8], mybir.dt.int16, kind="ExternalInput")
    ohe_d = nc.dram_tensor("oh_e", [ttot, P, win], F8, kind="ExternalInput")
    ohd_d = nc.dram_tensor("oh_d", [ttot, win, P], F8, kind="ExternalInput")
    gmap_d = nc.dram_tensor("gmap", [nt, P, G], DT, kind="ExternalInput")
    b1_d = nc.dram_tensor("b1rep", [win, out_c], F32, kind="ExternalInput")
    b2_d = nc.dram_tensor("b2rep", [win, out_c], F32, kind="ExternalInput")
    bfc_d = nc.dram_tensor("bfc2", [P, 2], F32, kind="ExternalInput")
    cinv_d = nc.dram_tensor("cinv", [P, G], F32, kind="ExternalInput")
    y_d = nc.dram_tensor("y", [out_c, G], F32, kind="ExternalOutput")

    cin1 = nc.dram_tensor("cin1", [nc_nodes, ROWB], U8, kind="Internal")
    tab1 = nc.dram_tensor("tab1", [n_nodes, ROWB], U8, kind="Internal", addr_space="Shared")
    cin2 = nc.dram_tensor("cin2", [nc_nodes, ROWB], U8, kind="Internal")
    tab2 = nc.dram_tensor("tab2", [n_nodes, ROWB], U8, kind="Internal", addr_space="Shared")
    pin = nc.dram_tensor("pin", [out_c, G], F32, kind="Internal")
    pout = nc.dram_tensor("pout", [out_c, G], F32, kind="Internal", addr_space="Shared")

    groups = [list(range(ncores))]

    with tile.TileContext(nc) as tc:
        with (
            tc.tile_pool(name="const", bufs=1) as cpool,
            tc.tile_pool(name="work", bufs=3) as wpool,
            tc.tile_pool(name="gath", bufs=4) as gpool,
            tc.tile_pool(name="rhsp", bufs=3) as rpool,
            tc.tile_pool(name="np", bufs=2, space="PSUM") as npp,
            tc.tile_pool(name="agg", bufs=2, space="PSUM") as aggp,
            tc.tile_pool(name="adp", bufs=1, space="PSUM") as adp,
            tc.tile_pool(name="trp", bufs=1, space="PSUM") as trp,
            tc.tile_pool(name="plp", bufs=1, space="PSUM") as plp,
        ):
            nc.gpsimd.load_library(mlp)
            # ---- constants ----
            ident = cpool.tile([P, P], DT)
            make_identity(nc, ident[:])
            w1_sb = cpool.tile([P, 2, TBLC], DT)
            nc.sync.dma_start(out=w1_sb[:, :, :], in_=w1_d.ap().rearrange("(kh p) m -> p kh m", p=P))
            w2_sb = cpool.tile([P, 2, TBLC], DT)
            nc.sync.dma_start(out=w2_sb[:, :, :], in_=w2_d.ap().rearrange("(kh p) m -> p kh m", p=P))
            wfc_sb = cpool.tile([P, 2, 2, P], DT)  # [k-half, m-half]
            nc.sync.dma_start(out=wfc_sb[:, :, :, :],
                              in_=wfc_d.ap().rearrange("(kh p) (mh q) -> p kh mh q", p=P, q=P))
            b1_sb = cpool.tile([win, out_c], F32)
            nc.sync.dma_start(out=b1_sb[:, :], in_=b1_d[:, :])
            b2_sb = cpool.tile([win, out_c], F32)
            nc.sync.dma_start(out=b2_sb[:, :], in_=b2_d[:, :])
            bfc_sb = cpool.tile([P, 2], F32)
            nc.sync.dma_start(out=bfc_sb[:, :], in_=bfc_d[:, :])
            cinv_sb = cpool.tile([P, G], F32)
            nc.sync.dma_start(out=cinv_sb[:, :], in_=cinv_d[:, :])
            isb = cpool.tile([P, ttot * 8], mybir.dt.int16)
            nc.sync.dma_start(out=isb[:, :], in_=idx_d[:, :])

            # ---- node phase ----
            # chunk bookkeeping for interleaved AllGathers
            chunk_end_tile = {}
            chunk_ob = []
            ob = 0
            for k, (lo, hi) in enumerate(bounds):
                chunk_end_tile[(hi + P - 1) // P - 1] = k
                chunk_ob.append(ob)
                ob += ncores * (hi - lo)

            def ag_chunk(cin, tab, k):
                # the collective's DMA is descriptor-bound: view the
                # contiguous row range as fat rows (fewer descriptors)
                lo, hi = bounds[k]
                s = hi - lo
                g0 = chunk_ob[k]
                p = 512
                while p < 8192 and (s * ROWB) % (2 * p) == 0:
                    p *= 2
                nc.gpsimd.collective_compute(
                    "AllGather", mybir.AluOpType.bypass,
                    ins=[cin.ap()[lo:hi, :].rearrange("r b -> (r b)")
                         .rearrange("(x p) -> x p", p=p)],
                    outs=[tab.ap()[g0:g0 + ncores * s, :].rearrange("r b -> (r b)")
                          .rearrange("(x p) -> x p", p=p)],
                    replica_groups=groups)

            def node_tile(t, lhsT_of, w_sb, ocols, cin):
                rows = min(P, nc_nodes - t * P)
                h2 = ocols - 256  # a-val cols (2*H)
                ps = npp.tile([P, TBLC], F32, tag="nps", name="nps")
                for kh in range(2):
                    nc.tensor.matmul(out=ps[:rows, :ocols], lhsT=lhsT_of(t, kh, rows),
                                     rhs=w_sb[:, kh, :ocols], start=(kh == 0), stop=(kh == 1))
                # row bytes: [h fp8 0:256 | one fp8 256 | pad | a_src bf16 258: | a_dst]
                hf8 = wpool.tile([P, 257], F8, tag="hf8", name="hf8")
                nc.vector.tensor_copy(out=hf8[:rows, 0:256], in_=ps[:rows, 0:256])
                nc.vector.memset(hf8[:rows, 256:257], 1.0)
                av = wpool.tile([P, 8], DT, tag="av", name="av")
                nc.vector.tensor_copy(out=av[:rows, :h2], in_=ps[:rows, 256:256 + h2])
                nc.sync.dma_start(out=cin.ap()[t * P:t * P + rows, 0:257],
                                  in_=hf8[:rows, :].bitcast(U8))
                nc.sync.dma_start(out=cin.ap()[t * P:t * P + rows, 258:258 + 2 * h2],
                                  in_=av[:rows, :h2].bitcast(U8))

            def node_phase(lhsT_of, w_sb, ocols, cin, tab=None):
                for t in range(nt):
                    node_tile(t, lhsT_of, w_sb, ocols, cin)
                    if tab is not None and t in chunk_end_tile:
                        ag_chunk(cin, tab, chunk_end_tile[t])

            # layer-1 node phase: xT staged in two bulk DMAs, sliced per tile
            xt_sb = cpool.tile([P, 2, nt * P], DT)
            xstep = ((nt + 4) // 5) * P
            for lo in range(0, nt * P, xstep):
                hi = min(lo + xstep, nt * P)
                for kh in range(2):
                    nc.sync.dma_start(out=xt_sb[:, kh, lo:hi], in_=xt_d[kh * P:(kh + 1) * P, lo:hi])
            def l1_lhsT(t, kh, rows):
                return xt_sb[:, kh, t * P:t * P + rows]
            node_phase(l1_lhsT, w1_sb, 256 + 2 * HEADS, cin1, tab=tab1)

            # ---- edge phase ----
            def edge_phase(tab, cin, brep, H, pool_into=None, after_window=None,
                           direct=False):
                """pool_into: optional psum pair — accumulate graph-pool matmuls
                from the relu output instead of writing it to DRAM.
                direct (H==1): exp-scaled onehot lhsT x raw fp8 gathered rhs;
                the ones byte at row offset 256 yields the softmax denominator
                in the same matmul (out col 256)."""
                RH = (out_c + 1) if direct else (H + out_c)
                ab0 = 258            # a_src byte offset in table row
                db0 = 258 + 2 * H    # a_dst byte offset
                for w in range(nwin):
                    size = min(win, nc_nodes - w * win)
                    lo_t, hi_t, base = int(tlo[w]), int(thi[w]), int(t0[w])
                    tw = lo_t + hi_t
                    s0 = base * P
                    # own-dst attention coefs
                    ad = wpool.tile([win, HEADS], DT, tag="adst")
                    nc.sync.dma_start(out=ad[:size, :H],
                                      in_=cin.ap()[w * win:w * win + size, db0:db0 + 2 * H].bitcast(DT))
                    # onehots for this window (fp8: 0/1 exact)
                    ohe = wpool.tile([P, twmax, win], F8, tag="ohe")
                    nc.sync.dma_start(out=ohe[:, :tw, :],
                                      in_=ohe_d.ap()[base:base + tw, :, :].rearrange("t p d -> p t d"))
                    ohd = wpool.tile([win, twmax, P], F8, tag="ohd")
                    nc.sync.dma_start(out=ohd[:, :tw, :],
                                      in_=ohd_d.ap()[base:base + tw, :, :].rearrange("t d e -> d t e"))
                    # gather source rows: batched SWDGE gather per table half,
                    # chunked to <=1024 indices per call (HW ucode limit)
                    g = gpool.tile([P, twmax, ROWB], U8, tag="g")
                    for tb, tn, ta in ((0, lo_t, tab.ap()[0:SPLIT, :]),
                                       (lo_t, hi_t, tab.ap()[SPLIT:n_nodes, :])):
                        for c0 in range(0, tn, 8):
                            cn = min(8, tn - c0)
                            K = cn * P
                            sa = s0 + (tb + c0) * P
                            nc.gpsimd.dma_gather(
                                g[:, tb + c0:tb + c0 + cn, :], ta,
                                isb[:, sa // 16:(sa + K) // 16], K, K, ROWB)
                    # a_dst expanded to edges: [128e, tw*H]
                    adps = adp.tile([P, twmax * HEADS], F32, tag="adps")
                    for t in range(tw):
                        nc.tensor.matmul(out=adps[:, t * H:(t + 1) * H],
                                         lhsT=ohd[:size, t, :], rhs=ad[:size, :H],
                                         start=True, stop=True)
                    # logits -> exp(leaky_relu)
                    lg = wpool.tile([P, twmax * HEADS], F32, tag="lg")
                    lg3 = lg[:, :tw * H].rearrange("p (t h) -> p t h", t=tw)
                    ad3 = adps[:, :tw * H].rearrange("p (t h) -> p t h", t=tw)
                    nc.vector.tensor_tensor(out=lg3, in0=g[:, :tw, ab0:ab0 + 2 * H].bitcast(DT),
                                            in1=ad3, op=OP.add)
                    lk = wpool.tile([P, twmax * HEADS], F32, tag="lk")
                    nc.vector.scalar_tensor_tensor(out=lk[:, :tw * H], in0=lg[:, :tw * H],
                                                   scalar=NEG_SLOPE, in1=lg[:, :tw * H],
                                                   op0=OP.mult, op1=OP.max)
                    ag = aggp.tile([win, RH], F32, tag="ag")
                    if direct:
                        # exp-scaled onehot; rhs is raw gathered fp8 [h | one]
                        et = wpool.tile([P, twmax], DT, tag="et")
                        nc.scalar.activation(out=et[:, :tw], in_=lk[:, :tw], func=AF.Exp)
                        ohs = rpool.tile([P, twmax, win], F8, tag="ohs")
                        nc.vector.tensor_tensor(out=ohs[:, :tw, :], in0=ohe[:, :tw, :],
                                                in1=et[:, :tw].to_broadcast([P, tw, win]),
                                                op=OP.mult)
                        for t in range(tw):
                            nc.tensor.matmul(out=ag[:, :], lhsT=ohs[:, t, :],
                                             rhs=g[:, t, 0:RH].bitcast(F8),
                                             start=(t == 0), stop=(t == tw - 1))
                    else:
                        rhs = rpool.tile([P, twmax, RH], DT, tag="rhs")
                        nc.scalar.activation(out=rhs[:, :tw, 0:H],
                                             in_=lk[:, :tw * H].rearrange("p (t h) -> p t h", t=tw),
                                             func=AF.Exp)
                        # rhs features = exp * h_src, one clean 3-d op per head
                        for h in range(H):
                            nc.vector.tensor_tensor(
                                out=rhs[:, :tw, H + h * 64:H + (h + 1) * 64],
                                in0=g[:, :tw, h * 64:(h + 1) * 64].bitcast(F8),
                                in1=rhs[:, :tw, h:h + 1].to_broadcast([P, tw, 64]),
                                op=OP.mult)
                        for t in range(tw):
                            nc.tensor.matmul(out=ag[:, :], lhsT=ohe[:, t, :], rhs=rhs[:, t, :],
                                             start=(t == 0), stop=(t == tw - 1))
                    # normalize + bias + relu
                    dcol = out_c if direct else 0  # denominator column base in ag
                    fcol = 0 if direct else H      # feature column base
                    s = wpool.tile([win, HEADS], F32, tag="s")
                    nc.vector.tensor_scalar_max(s[:size, :H], ag[:size, dcol:dcol + H], 1e-30)
                    nc.vector.reciprocal(out=s[:size, :H], in_=s[:size, :H])
                    on = wpool.tile([win, out_c], F32, tag="on")
                    nc.vector.tensor_tensor(
                        out=on[:size, :].rearrange("d (h c) -> d h c", h=H),
                        in0=ag[:size, fcol:fcol + out_c].rearrange("d (h c) -> d h c", h=H),
                        in1=s[:size, :H].to_broadcast([size, H, out_c // H]), op=OP.mult)
                    nc.vector.tensor_tensor(out=on[:size, :], in0=on[:size, :],
                                            in1=brep[:size, :], op=OP.add)
                    ro = wpool.tile([win, out_c], DT, tag="ro")
                    nc.scalar.activation(out=ro[:size, :], in_=on[:size, :], func=AF.Relu)
                    if after_window is not None:
                        after_window(w, ro, size)
                    if pool_into is not None:
                        gm = wpool.tile([P, G], DT, tag="gm")
                        nc.sync.dma_start(out=gm[:, :], in_=gmap_d[w, :, :])
                        for mh in range(2):
                            nc.tensor.matmul(out=pool_into[mh][:, :],
                                             lhsT=ro[:size, mh * P:(mh + 1) * P],
                                             rhs=gm[:size, :],
                                             start=(w == 0), stop=(w == nwin - 1))

            def l1_after_window(w, ro, size):
                def l2_lhsT(t, kh, rows):
                    tp = trp.tile([P, P], DT, tag="tp", name="tp")
                    nc.tensor.transpose(out=tp[:, :rows], in_=ro[:rows, kh * P:(kh + 1) * P],
                                        identity=ident[:rows, :rows])
                    tl = wpool.tile([P, P], DT, tag="tl", name="tl")
                    nc.vector.tensor_copy(out=tl[:, :rows], in_=tp[:, :rows])
                    return tl[:, :rows]
                node_tile(w, l2_lhsT, w2_sb, 258, cin2)
                if w in chunk_end_tile:
                    ag_chunk(cin2, tab2, chunk_end_tile[w])

            edge_phase(tab1, cin1, b1_sb, HEADS, after_window=l1_after_window)

            assert win == P and nwin == nt
            pps = [plp.tile([P, G], F32, tag=f"pp{mh}", name=f"pp{mh}") for mh in range(2)]
            edge_phase(tab2, cin2, b2_sb, 1, pool_into=pps, direct=True)

            # ---- pool + fc ----
            psb = wpool.tile([P, 2, G], F32, tag="psb")
            for mh in range(2):
                nc.vector.tensor_copy(out=psb[:, mh, :], in_=pps[mh][:, :])
            nc.sync.dma_start(out=pin.ap().rearrange("(mh p) g -> p mh g", p=P), in_=psb[:, :, :])

            nc.gpsimd.collective_compute(
                "AllReduce", mybir.AluOpType.add,
                ins=[pin.ap()], outs=[pout.ap()], replica_groups=groups)

            pr = wpool.tile([P, 2, G], F32, tag="pr")
            nc.sync.dma_start(out=pr[:, :, :], in_=pout.ap().rearrange("(mh p) g -> p mh g", p=P))
            pm = wpool.tile([P, 2, G], DT, tag="pm")
            for kh in range(2):
                nc.vector.tensor_tensor(out=pm[:, kh, :], in0=pr[:, kh, :], in1=cinv_sb[:, :], op=OP.mult)
            for mh in range(2):
                fps = aggp.tile([P, G], F32, tag="ag")
                for kh in range(2):
                    nc.tensor.matmul(out=fps[:, :], lhsT=wfc_sb[:, kh, mh, :], rhs=pm[:, kh, :],
                                     start=(kh == 0), stop=(kh == 1))
                yo = wpool.tile([P, G], F32, tag="yo")
                nc.scalar.activation(out=yo[:, :], in_=fps[:, :], func=AF.Relu,
                                     bias=bfc_sb[:, mh:mh + 1], scale=1.0)
                nc.sync.dma_start(out=y_d[mh * P:(mh + 1) * P, :], in_=yo[:, :])

    # Spread gathers over the 4 SWDGE queues for parallel descriptor
    # generation. Tile sem assignment rotates the 8 DMASW lanes over Pool DMA
    # instructions in scheduled (block) order; queue = lane % 4 keeps each
    # lane pinned to one queue.
    k = 0
    for b in nc.main_func.blocks:
        for i in b.instructions:
            if isinstance(i, mybir.InstDMAGatherAnt):
                i.queue_num = (k % 8) % 4
                k += 1

    nc.compile()
    return nc


def _install_ntff_hook():
    """Register the NTFF profile hook (the image's antenv lacks axon_hooks)."""
    import types
    mod = sys.modules.get("antenv.axon_hooks")
    if mod is None:
        import antenv
        mod = types.ModuleType("antenv.axon_hooks")
        mod._hook = None
        mod.set_axon_ntff_profile_hook = lambda h: setattr(mod, "_hook", h)
        mod.get_axon_ntff_profile_hook = lambda: mod._hook
        sys.modules["antenv.axon_hooks"] = mod
        antenv.axon_hooks = mod
    if mod._hook is None:
        from trn_agent_boot.trn_boot import _ntff_profile_via_ctypes
        mod.set_axon_ntff_profile_hook(_ntff_profile_via_ctypes("/opt/axon/libaxon_pjrt.so"))

# --------------------------------------------------------------------------
# entry point
# --------------------------------------------------------------------------

def kernel(**inputs) -> np.ndarray:
    global LAST_EXEC_NS
    from concourse.bass_utils import run_bass_kernel_spmd

    args = {k: np.asarray(v) for k, v in inputs.items()}
    perm = balance_nodes(args["edge_index"][1], N_NODES, NCORES, WIN)
    old_of_new = np.argsort(perm)
    args["x"] = args["x"][old_of_new]
    args["batch"] = args["batch"][old_of_new]
    ei = args["edge_index"]
    args["edge_index"] = np.stack([perm[ei[0]], perm[ei[1]]]).astype(ei.dtype)
    tlo, thi, t0, per_core = build_host_inputs(
        args["x"], args["edge_index"], args["batch"],
        args["W1"], args["att_src1"], args["att_dst1"], args["b1"],
        args["W2"], args["att_src2"], args["att_dst2"], args["b2"],
        args["Wfc"], args["bfc"],
        N_NODES, N_GRAPHS, NCORES, WIN)
    nc = build_program(tlo, thi, t0, N_NODES, N_GRAPHS, NCORES, WIN)

    trace = os.environ.get("GAT_TRACE") == "1"
    if trace:
        try:
            _install_ntff_hook()
        except Exception:
            trace = False
    res = run_bass_kernel_spmd(nc, per_core, core_ids=list(range(NCORES)), trace=trace)
    LAST_EXEC_NS = res.exec_time_ns
    y = res.results[0]["y"]
    return np.ascontiguousarray(y.T).astype(np.float32)
